# revision 1
# baseline (speedup 1.0000x reference)
"""MoE (top-2 of 8 experts, SwiGLU) Trainium2 kernel - sparse expert-parallel.

kernel(**inputs) takes the FULL inputs (x [2,1024,1024] f32, router_w
[1024,8] f32, w1/w3 [8,1024,2048] f32, w2 [8,2048,1024] f32, top_k=2) and
returns the FULL output [2,1024,1024] f32. Internally it shards
expert-parallel across the 8 NeuronCores: core e holds expert e's weights
(cast to bf16 on host) plus the full token set, and runs one SPMD Bass/Tile
program:

  1. fp32 router on all T=2048 tokens in x^T layout (k-outer accumulation
     over two 8-tile PSUM waves so routing overlaps the xT DMA), then one
     batched softmax/top-2 pass over [128, 16, 8] with free-dim broadcasts.
     sel in S[:,i], gate (= prob if selected else 0) in gsb[:,i].
  2. On-chip compaction: exclusive prefix positions via two triangular
     matmuls (within-tile prefix + per-tile counts) and a 4-step shift-add
     scan across tiles; unselected tokens go to a trash zone (>= 4096).
  3. x is cast-DMA'd to bf16 during routing and scaled in place by the fp32
     gate; a 0/1 selection matrix P_sel[:,k,s] = (pos[:,k] == s) (bf16,
     exact) turns the token gather into PE matmuls:
     xgT[d, s] = sum_k xs_k^T P_sel_k, with capacity C=640 slots.
  4. bf16 SwiGLU MLP over the C slots (fp32 PSUM accumulation), chunked
     [384, 256] so gather -> L1 -> L2 pipeline per chunk; layer 2 uses h^T
     as the stationary operand to emit slot-major y_g [C, 1024] directly.
  5. Outputs per core: compact y_g + pos [128,16]. Host combine:
     out[t] += y_g[pos[t]] for pos[t] < C; tokens dropped by capacity
     overflow (pos in [C, 4096), ~impossible for balanced routing) are
     recomputed exactly on host in numpy.

Measured on 8 axon-attached TRN2 NeuronCores: ~220 us/exec (For_i reps=64
slope), relative error 5.2e-3 vs the fp32 reference (bf16 MLP precision).
"""

import numpy as np
import ml_dtypes

import concourse.bass as bass
import concourse.bacc as bacc
import concourse.mybir as mybir
import concourse.tile as tile
from concourse.bass import ts
from concourse.bass_utils import run_bass_kernel_spmd

P = 128
T = 2048
D = 1024
H = 2048
E = 8
KD = D // P
KH = H // P
TT = T // P
C = 576
CT = C // P
CHUNKS = [(0, 384), (384, 192)]  # (start, width)
NC = 512
TRASH = 4096   # trash-zone base for unselected tokens (> C + T)

F32 = mybir.dt.float32
BF16 = mybir.dt.bfloat16
I32 = mybir.dt.int32
AX = mybir.AxisListType
ALU = mybir.AluOpType
ACTF = mybir.ActivationFunctionType


def _router(nc, tc, xf, xTsb, rwsb, eselsb, S, gsb):
    """Router matmuls (k-outer, 2 PSUM waves) + batched softmax/top-2."""
    LG = xf.tile([P, TT, E], F32, tag="LG", name="LG")
    with tc.tile_pool(name="rps", bufs=1, space="PSUM") as rps:
        for w in range(2):
            lgs = [
                rps.tile([P, E], F32, tag=f"lg{j}", name=f"lg{w}_{j}")
                for j in range(8)
            ]
            for k in range(KD):
                for j in range(8):
                    i = w * 8 + j
                    nc.tensor.matmul(
                        lgs[j],
                        lhsT=xTsb[:, k, ts(i, P)],
                        rhs=rwsb[:, k, :],
                        start=(k == 0),
                        stop=(k == KD - 1),
                    )
            for j in range(8):
                nc.scalar.activation(LG[:, w * 8 + j, :], lgs[j], ACTF.Copy)

    with tc.tile_pool(name="rsb", bufs=1) as rsb:
        def t3(tag):
            return rsb.tile([P, TT, E], F32, tag=tag, name=tag)

        def t2(tag):
            return rsb.tile([P, TT], F32, tag=tag, name=tag)

        def b(ap2):
            return ap2[:, :, None].broadcast_to([P, TT, E])

        mx = t2("mx")
        nc.vector.tensor_reduce(mx, LG, axis=AX.X, op=ALU.max)
        sh = t3("sh")
        nc.vector.tensor_tensor(sh, LG, b(mx), op=ALU.subtract)
        ex = t3("ex")
        nc.scalar.activation(ex, sh, ACTF.Exp)
        sm = t2("sm")
        nc.vector.tensor_reduce(sm, ex, axis=AX.X, op=ALU.add)
        rc = t2("rc")
        nc.vector.reciprocal(rc, sm)
        probs = t3("probs")
        nc.vector.tensor_tensor(probs, ex, b(rc), op=ALU.mult)
        m1 = t2("m1")
        nc.vector.tensor_reduce(m1, probs, axis=AX.X, op=ALU.max)
        mask = t3("mask")
        nc.vector.tensor_tensor(mask, probs, b(m1), op=ALU.is_ge)
        masked = t3("masked")
        nc.vector.scalar_tensor_tensor(
            masked, mask, -1e30, probs, op0=ALU.mult, op1=ALU.add
        )
        m2 = t2("m2")
        nc.vector.tensor_reduce(m2, masked, axis=AX.X, op=ALU.max)
        pse = t3("pse")
        nc.vector.tensor_tensor(
            pse, probs, eselsb[:, None, :].broadcast_to([P, TT, E]), op=ALU.mult
        )
        pex = t2("pex")
        nc.vector.tensor_reduce(pex, pse, axis=AX.X, op=ALU.add)
        nc.vector.tensor_tensor(S, pex, m2, op=ALU.is_ge)
        nc.vector.tensor_tensor(gsb, pex, S, op=ALU.mult)


def _compaction(nc, tc, S, cpt, ltri, onesm, posf, pos_h):
    with (
        tc.tile_pool(name="cps", bufs=1, space="PSUM") as cps,
        tc.tile_pool(name="csb", bufs=1) as csb,
    ):
        Aps = cps.tile([P, TT], F32, tag="A", name="Aps")
        nc.tensor.matmul(Aps, lhsT=ltri, rhs=S, start=True, stop=True)
        Bps = cps.tile([P, TT], F32, tag="B", name="Bps")
        nc.tensor.matmul(Bps, lhsT=onesm, rhs=S, start=True, stop=True)
        W0 = csb.tile([P, TT], F32, tag="W0", name="W0")
        nc.vector.tensor_copy(W0, Bps)
        Wp = W0
        for sft in (1, 2, 4, 8):
            Wn = csb.tile([P, TT], F32, tag=f"W{sft}", name=f"W{sft}")
            nc.vector.tensor_copy(Wn[:, 0:sft], Wp[:, 0:sft])
            nc.vector.tensor_tensor(
                Wn[:, sft:TT], Wp[:, sft:TT], Wp[:, 0 : TT - sft], op=ALU.add
            )
            Wp = Wn
        nc.vector.tensor_tensor(posf, Wp, Bps, op=ALU.subtract)
        nc.vector.tensor_tensor(posf, posf, Aps, op=ALU.add)
        nc.vector.tensor_tensor(posf, posf, cpt, op=ALU.subtract)
        nc.vector.tensor_tensor(posf, posf, S, op=ALU.mult)
        nc.vector.tensor_tensor(posf, posf, cpt, op=ALU.add)
        pos_i32 = csb.tile([P, TT], I32, tag="posi", name="pos_i32")
        nc.vector.tensor_copy(pos_i32, posf)
        nc.sync.dma_start(pos_h, pos_i32)


def build_moe_nc(reps=1):
    nc = bacc.Bacc("TRN2", target_bir_lowering=False, debug=False)

    xT_h = nc.dram_tensor("xT", [D, T], F32, kind="ExternalInput").ap()
    x_h = nc.dram_tensor("x", [T, D], F32, kind="ExternalInput").ap()
    rw_h = nc.dram_tensor("rw", [D, E], F32, kind="ExternalInput").ap()
    esel_h = nc.dram_tensor("esel", [P, E], F32, kind="ExternalInput").ap()
    ltri_h = nc.dram_tensor("ltri", [P, P], F32, kind="ExternalInput").ap()
    onesm_h = nc.dram_tensor("onesm", [P, P], F32, kind="ExternalInput").ap()
    cpt_h = nc.dram_tensor("cpt", [P, TT], F32, kind="ExternalInput").ap()
    iotac_h = nc.dram_tensor("iotac", [P, C], F32, kind="ExternalInput").ap()
    w1_h = nc.dram_tensor("w1", [D, H], BF16, kind="ExternalInput").ap()
    w3_h = nc.dram_tensor("w3", [D, H], BF16, kind="ExternalInput").ap()
    w2_h = nc.dram_tensor("w2", [H, D], BF16, kind="ExternalInput").ap()
    yg_h = nc.dram_tensor("y_g", [C, D], F32, kind="ExternalOutput").ap()
    pos_h = nc.dram_tensor("pos", [P, TT], I32, kind="ExternalOutput").ap()

    import contextlib

    with tile.TileContext(nc) as tc:
        hint = (
        mybir.EngineType.PE,
        mybir.EngineType.DVE,
        mybir.EngineType.Activation,
        mybir.EngineType.SP,
        mybir.EngineType.Pool,
    )
        loop_cm = (
            tc.For_i(0, reps, 1, hint_engines=hint)
            if reps > 1
            else contextlib.nullcontext()
        )
        with loop_cm, tc.tile_pool(name="wA", bufs=1) as wA:
            S = wA.tile([P, TT], F32, tag="S")
            gsb = wA.tile([P, TT], F32, tag="gsb")
            posf = wA.tile([P, TT], F32, tag="posf")
            ltri = wA.tile([P, P], F32, tag="ltri")
            onesm = wA.tile([P, P], F32, tag="onesm")
            cpt = wA.tile([P, TT], F32, tag="cpt")
            iotac = wA.tile([P, C], F32, tag="iotac")
            w1sb = wA.tile([P, KD, H], BF16, tag="w1")
            w3sb = wA.tile([P, KD, H], BF16, tag="w3")

            with tc.tile_pool(name="xup", bufs=1) as xup:
                xu = [
                    xup.tile([P, D], BF16, tag=f"xu{i}", name=f"xu{i}")
                    for i in range(TT)
                ]

                with tc.tile_pool(name="xf", bufs=1) as xf:
                    # DMA order: tiny router consts first, xT next
                    # (router-critical), then x cast, then w1/w3.
                    rwsb = xf.tile([P, KD, E], F32, tag="rw")
                    nc.sync.dma_start(rwsb, rw_h.rearrange("(k p) e -> p k e", p=P))
                    eselsb = xf.tile([P, E], F32, tag="esel")
                    nc.sync.dma_start(eselsb, esel_h)
                    nc.sync.dma_start(ltri, ltri_h)
                    nc.sync.dma_start(onesm, onesm_h)
                    nc.sync.dma_start(cpt, cpt_h)
                    nc.sync.dma_start(iotac, iotac_h)
                    xTsb = xf.tile([P, KD, T], F32, tag="xT")
                    for k in range(KD):
                        nc.sync.dma_start(xTsb[:, k, :], xT_h[ts(k, P), :])
                    for i in range(TT):
                        nc.gpsimd.dma_start(xu[i], x_h[ts(i, P), :])
                    for k in range(KD):
                        nc.sync.dma_start(w1sb[:, k, :], w1_h[ts(k, P), :])
                        nc.sync.dma_start(w3sb[:, k, :], w3_h[ts(k, P), :])

                    _router(nc, tc, xf, xTsb, rwsb, eselsb, S, gsb)
                    _compaction(nc, tc, S, cpt, ltri, onesm, posf, pos_h)
                # xf freed (xT space reusable)

                # fold gates into xu (in-place, per-partition fp32 scale);
                # half the tiles on ACT, half on DVE to halve the serial span
                for i in range(TT):
                    if i % 2 == 0:
                        nc.scalar.activation(
                            xu[i], xu[i], ACTF.Copy, scale=gsb[:, i : i + 1]
                        )
                    else:
                        nc.vector.tensor_scalar_mul(xu[i], xu[i], gsb[:, i : i + 1])

                with tc.tile_pool(name="hp1", bufs=1) as hp1:
                    xgT = [
                        hp1.tile([P, KD, w], BF16, tag=f"xgT{n}", name=f"xgT{n}")
                        for n, (st, w) in enumerate(CHUNKS)
                    ]
                    with (
                        tc.tile_pool(name="sp2", bufs=1) as sp2,
                        tc.tile_pool(name="ppg", bufs=2, space="PSUM") as ppg,
                    ):
                        psel = [
                            sp2.tile([P, C], BF16, tag=f"psel{i}", name=f"psel{i}")
                            for i in range(TT)
                        ]
                        for i in range(TT):
                            nc.vector.tensor_scalar(
                                psel[i],
                                iotac,
                                posf[:, i : i + 1],
                                None,
                                op0=ALU.is_equal,
                            )
                        for nch, (st, wd) in enumerate(CHUNKS):
                            for md in range(KD):
                                pg = ppg.tile([P, wd], F32, tag="pg", name="pg")
                                for k in range(TT):
                                    nc.tensor.matmul(
                                        pg,
                                        lhsT=xu[k][:, ts(md, P)],
                                        rhs=psel[k][:, st : st + wd],
                                        start=(k == 0),
                                        stop=(k == TT - 1),
                                    )
                                nc.vector.tensor_copy(xgT[nch][:, md, :], pg)

                    with (
                        tc.tile_pool(name="hp2", bufs=1) as hp2,
                        tc.tile_pool(name="mp", bufs=3) as mp,
                        tc.tile_pool(name="pp13", bufs=3, space="PSUM") as pp13,
                        tc.tile_pool(name="ppy", bufs=2, space="PSUM") as ppy,
                    ):
                        w2sb = hp2.tile([P, KH, D], BF16, tag="w2")
                        for k in range(KH):
                            nc.sync.dma_start(w2sb[:, k, :], w2_h[ts(k, P), :])
                        hT = [
                            hp2.tile([P, KH, w], BF16, tag=f"hT{n}", name=f"hT{n}")
                            for n, (st, w) in enumerate(CHUNKS)
                        ]
                        for nch, (st, wd) in enumerate(CHUNKS):
                            for m in range(KH):
                                ps1 = pp13.tile([P, wd], F32, tag="ps1", name="ps1")
                                ps3 = pp13.tile([P, wd], F32, tag="ps3", name="ps3")
                                for k in range(KD):
                                    nc.tensor.matmul(
                                        ps1,
                                        lhsT=w1sb[:, k, ts(m, P)],
                                        rhs=xgT[nch][:, k, :],
                                        start=(k == 0),
                                        stop=(k == KD - 1),
                                    )
                                for k in range(KD):
                                    nc.tensor.matmul(
                                        ps3,
                                        lhsT=w3sb[:, k, ts(m, P)],
                                        rhs=xgT[nch][:, k, :],
                                        start=(k == 0),
                                        stop=(k == KD - 1),
                                    )
                                sg = mp.tile([P, wd], BF16, tag="sg", name="sg")
                                nc.scalar.activation(sg, ps1, ACTF.Sigmoid)
                                u13 = mp.tile([P, wd], BF16, tag="u13", name="u13")
                                nc.vector.tensor_tensor(u13, sg, ps1, op=ALU.mult)
                                nc.vector.tensor_tensor(
                                    hT[nch][:, m, :], u13, ps3, op=ALU.mult
                                )
                            # full-128-row pieces only (M<128 matmul output is
                            # hardware-fatal); a non-multiple-of-128 chunk gets an
                            # overlapping last piece that recomputes identical rows
                            offs = list(range(0, wd - P + 1, P))
                            if offs[-1] != wd - P:
                                offs.append(wd - P)
                            for poff in offs:
                                row0 = st + poff
                                for n2 in range(D // NC):
                                    psy = ppy.tile([P, NC], F32, tag="psy", name="psy")
                                    for k in range(KH):
                                        nc.tensor.matmul(
                                            psy,
                                            lhsT=hT[nch][:, k, poff : poff + P],
                                            rhs=w2sb[:, k, ts(n2, NC)],
                                            start=(k == 0),
                                            stop=(k == KH - 1),
                                        )
                                    ysb = mp.tile([P, NC], F32, tag="ysb", name="ysb")
                                    nc.scalar.activation(ysb, psy, ACTF.Copy)
                                    nc.sync.dma_start(
                                        yg_h[row0 : row0 + P, ts(n2, NC)], ysb
                                    )
    nc.compile()
    return nc


_NC_CACHE = None


def _get_nc():
    global _NC_CACHE
    if _NC_CACHE is None:
        _NC_CACHE = build_moe_nc()
    return _NC_CACHE


def make_in_maps(x, router_w, w1, w2, w3):
    xt = np.ascontiguousarray(np.asarray(x, np.float32).reshape(T, D))
    xT = np.ascontiguousarray(xt.T)
    rw = np.ascontiguousarray(np.asarray(router_w, np.float32))
    w1b = np.asarray(w1).astype(ml_dtypes.bfloat16)
    w2b = np.asarray(w2).astype(ml_dtypes.bfloat16)
    w3b = np.asarray(w3).astype(ml_dtypes.bfloat16)
    ltri = np.triu(np.ones((P, P), np.float32), k=1)
    onesm = np.ones((P, P), np.float32)
    iota = (np.arange(TT)[None, :] * P + np.arange(P)[:, None]).astype(np.int32)
    cpt = (TRASH + iota).astype(np.float32)
    iotac = np.broadcast_to(np.arange(C, dtype=np.float32), (P, C)).copy()
    in_maps = []
    for e in range(E):
        esel = np.zeros((P, E), np.float32)
        esel[:, e] = 1.0
        in_maps.append(
            {
                "xT": xT,
                "x": xt,
                "rw": rw,
                "esel": esel,
                "ltri": ltri,
                "onesm": onesm,
                "cpt": cpt,
                "iotac": iotac,
                "w1": np.ascontiguousarray(w1b[e]),
                "w3": np.ascontiguousarray(w3b[e]),
                "w2": np.ascontiguousarray(w2b[e]),
            }
        )
    return in_maps


def combine(results, inputs=None):
    """Sum per-core compact outputs back to token positions.

    pos semantics: < C -> kept at that slot; in [C, TRASH) -> selected but
    dropped (capacity overflow; recompute on host); >= TRASH -> not selected.
    """
    out = np.zeros((T, D), np.float32)
    t_idx = (np.arange(TT)[None, :] * P + np.arange(P)[:, None]).astype(np.int64)
    for e in range(E):
        posv = results[e]["pos"].astype(np.int64)
        selm = posv < C
        out[t_idx[selm]] += results[e]["y_g"][posv[selm]]
        dropm = (posv >= C) & (posv < TRASH)
        if dropm.any() and inputs is not None:
            xt = np.asarray(inputs["x"], np.float32).reshape(T, D)
            rw = np.asarray(inputs["router_w"], np.float32)
            for t in t_idx[dropm]:
                lg = xt[t] @ rw
                p = np.exp(lg - lg.max())
                p /= p.sum()
                xs = xt[t] * p[e]
                a1 = xs @ np.asarray(inputs["w1"][e], np.float32)
                a3 = xs @ np.asarray(inputs["w3"][e], np.float32)
                h = (a1 / (1 + np.exp(-a1))) * a3
                out[t] += h @ np.asarray(inputs["w2"][e], np.float32)
    return out


def kernel(x, router_w, w1, w2, w3, top_k):
    assert int(top_k) == 2
    nc = _get_nc()
    in_maps = make_in_maps(x, router_w, w1, w2, w3)
    res = run_bass_kernel_spmd(nc, in_maps, list(range(E))).results
    inputs = dict(x=x, router_w=router_w, w1=w1, w2=w2, w3=w3)
    return combine(res, inputs).reshape(2, T // 2, D)

